# revision 34
# baseline (speedup 1.0000x reference)
"""Causal multi-head attention (B=1, N=4096, H=16, D=64) on 8 trn2 cores.

Sharding: head-parallel tensor parallelism - 2 heads per core.  Each core
reads the full x (pre-transposed on host), computes QKV for its 2 heads,
runs causal flash-style attention in the S^T (k-partition, q-free) layout,
applies its 128-column slice of the output projection, and writes a partial
[4096, 1024] y.  Host sums the 8 partials and adds b_proj.

Fused pipeline: QKV projection for sequence chunk j feeds the attention
q-block j immediately (causality means block j only needs K/V chunks 0..j),
and the next chunk's QKV chains are emitted into the previous q-block's exp
drain so the PE never heads on the ACT engine.  Causal masking is a
multiplicative 0/1 triangle on the diagonal 128-bands applied post-exp on
the DVE; the softmax denominator (accumulated via a ones-column of V') is
broadcast across partitions with a rank-1 PE matmul; partial y is written
in bf16 (the host accumulates in f32).

Environment notes (bisected on HW): gpsimd/Pool instructions, XBAR DMA
transpose, software-DGE DMAs, 3D (non-2D) HWDGE DMA patterns, and
custom-DVE ops reading PSUM all crash this axon image - the K_* env flags
default to the safe paths.
"""

import numpy as np

import concourse.bass as bass
from concourse import bacc
import concourse.tile as tile
from concourse import mybir
from concourse.bass_utils import run_bass_kernel_spmd

B, N, H, D = 1, 4096, 16, 64
C = H * D  # 1024
SCALE = D ** -0.5
NCORES = 8
HPC = H // NCORES  # heads per core = 2
F32 = mybir.dt.float32

# matmul operand dtype: "bf16" (1 cyc/row, ~1e-3 rel err), "f32r" (2 cyc/row,
# ~2e-4), "f32" (4 cyc/row, ~1e-6)
MM_MODE = "bf16"
MMDT = {"bf16": mybir.dt.bfloat16, "f32r": mybir.dt.float32r,
        "f32": mybir.dt.float32}[MM_MODE]

NKT = N // 128      # 32 k-tiles of 128
NQB = N // 512      # 8 q-blocks of 512
NCT = C // 128      # 8 contraction tiles for the projections

import os
USE_XBAR = os.environ.get("K_XBAR", "0") == "1"      # XBAR dma transpose for V'
USE_AFFINE = os.environ.get("K_AFFINE", "0") == "1"  # gpsimd affine_select mask
USE_GPDMA = os.environ.get("K_GPDMA", "0") == "1"    # gpsimd-issued small DMAs
USE_BATCHDMA = os.environ.get("K_BATCHDMA", "0") == "1"  # 3D batched loads
USE_RECIP_PRE = os.environ.get("K_RECIP2", "0") == "1"   # recip before broadcast


def build_nc():
    nc = bacc.Bacc("TRN2", target_bir_lowering=False)

    xT = nc.dram_tensor("xT", [128, NQB, NCT, 512], MMDT,
                        kind="ExternalInput").ap()
    wqk = nc.dram_tensor("wqk", [128, NCT, 256], MMDT,
                         kind="ExternalInput").ap()
    wv = nc.dram_tensor("wv", [128, NCT, 128], MMDT,
                        kind="ExternalInput").ap()
    wp = nc.dram_tensor("wp", [128, C], MMDT, kind="ExternalInput").ap()
    amask = (None if USE_AFFINE else
             nc.dram_tensor("amask", [128, 128], F32, kind="ExternalInput").ap())
    ident = (None if USE_XBAR else
             nc.dram_tensor("ident", [128, 128], F32, kind="ExternalInput").ap())
    y = nc.dram_tensor("y", [N, C], MMDT, kind="ExternalOutput").ap()

    with tile.TileContext(nc) as tc:
        _body(tc, xT, wqk, wv, wp, amask, ident, y)
    nc.compile()
    return nc


def _qkv_pieces(tc, j, xt, mm, Wqk, Wv, QT, KT, VT, VP, ident_sb):
    """QKV projection for chunk j as a list of emit-closures, so chains can
    be interleaved into the previous q-block's exp drain."""
    nc = tc.nc

    def proj(og, dest):
        def emit():
            ps = mm.tile([128, 512], F32, tag="mm", name=f"qkv_{j}_{og}")
            for ct in range(NCT):
                if og < 2:
                    lhsT = Wqk[:, ct, 128 * og : 128 * og + 128]
                else:
                    lhsT = Wv[:, ct, :]
                nc.tensor.matmul(
                    ps, lhsT, xt[:, ct, :],
                    start=(ct == 0), stop=(ct == NCT - 1),
                )
            nc.vector.tensor_copy(dest[:, 512 * j : 512 * (j + 1)], ps)
        return emit

    def vtrans():
        # V^T -> V' transpose, 4 k-tiles per chunk
        if USE_XBAR:
            for s in range(4):
                kt = 4 * j + s
                for h in range(2):
                    nc.sync.dma_start_transpose(
                        out=VP[:, kt, h, 0:64],
                        in_=VT[64 * h : 64 * h + 64, 128 * kt : 128 * (kt + 1)],
                    )
        else:
            trp = mm.tile([128, 512], F32, tag="mm", name=f"tr_{j}")
            for s in range(4):
                kt = 4 * j + s
                nc.tensor.transpose(
                    trp[:, 128 * s : 128 * (s + 1)],
                    VT[:, 128 * kt : 128 * (kt + 1)], ident_sb,
                )
                nc.vector.tensor_copy(
                    VP[:, kt, :, 0:64],
                    trp[:, 128 * s : 128 * (s + 1)].rearrange(
                        "p (g c) -> p g c", g=2),
                )

    return [proj(0, QT), proj(1, KT), proj(2, VT), vtrans]


def _attention_qb(tc, qb, spool, opool, ptpool, QT, KT, VP, o_ps, zero_reg,
                  amask_sb, tail_work=(), mid_work=()):
    """Causal attention for q-block qb in the S^T layout.

    tail_work: closures for the next chunk's QKV; the first half is emitted
    after the third-from-last group's AVs, the rest between the last group's
    exps and its AV matmuls, so the PE has ready work while the exp tail
    drains on ACT without over-delaying the epilogue.
    mid_work: closures emitted after the first group's AVs (for work that
    needs a short head start on other engines, like the previous q-block's
    output projection waiting on its 1/l round-trip).
    """
    nc = tc.nc
    Exp = mybir.ActivationFunctionType.Exp
    n_kt = 4 * (qb + 1)  # causal: k-tiles 0 .. 4qb+3
    q0 = 512 * qb
    n_g = n_kt // 2

    for g in range(n_g):
        last = g == n_g - 1          # k-tiles jr=2,3 (diagonal pair)
        penult = g == n_g - 2        # k-tiles jr=0,1
        # one 1-bank s tile per (h, i): 4-deep ring keeps the exp stream on
        # the ACT engine back-to-back instead of ping-ponging with the PE
        s_ps = {}
        pt = {}
        cs = []
        for i in range(2):
            kt = 2 * g + i
            jr = kt - 4 * qb
            cs.append(128 * jr if jr > 0 else 0)
        for h in range(2):
            for i in range(2):
                kt = 2 * g + i
                c0 = cs[i] if (last or penult) else 0
                s_ps[h, i] = spool.tile([128, 512], F32, tag="s",
                                        name=f"s_{qb}_{g}_{h}_{i}")
                pt[h, i] = ptpool.tile([128, 512], MMDT, tag="pt",
                                       name=f"pt_{qb}_{g}_{h}_{i}")
                nc.tensor.matmul(
                    s_ps[h, i][:, c0:512],
                    KT[64 * h : 64 * h + 64, 128 * kt : 128 * (kt + 1)],
                    QT[64 * h : 64 * h + 64, q0 + c0 : q0 + 512],
                    start=True, stop=True,
                )
        for h in range(2):
            for i in range(2):
                c0 = cs[i] if (last or penult) else 0
                nc.scalar.activation(
                    pt[h, i][:, c0:512], s_ps[h, i][:, c0:512],
                    Exp, scale=SCALE,
                )
                if last or penult:
                    # zero upper-triangular part of the diagonal 128-band:
                    # keep pt[p, cs+f] iff (q0+cs+f) >= (128kt+p) <=> f >= p
                    band = pt[h, i][:, cs[i] : cs[i] + 128]
                    if USE_AFFINE:
                        nc.gpsimd.affine_select(
                            out=band, in_=band,
                            pattern=[[1, 128]],
                            compare_op=mybir.AluOpType.is_ge,
                            fill=zero_reg,
                            base=0,
                            channel_multiplier=-1,
                        )
                    else:
                        nc.vector.tensor_mul(band, band, amask_sb)
        if g == max(0, n_g - 3) and n_g > 1:
            for work in tail_work[:2]:
                work()
        if g == n_g - 1:
            for work in (tail_work[2:] if n_g > 1 else tail_work):
                work()
        for h in range(2):
            for i in range(2):
                kt = 2 * g + i
                nc.tensor.matmul(
                    o_ps[h][:, cs[i]:512],
                    VP[:, kt, h, :],
                    pt[h, i][:, cs[i]:512],
                    start=(kt == 0), stop=(kt == n_kt - 1),
                )
        if g == 0:
            for work in mid_work:
                work()


def _epilogue_a(tc, qb, smsb, smsb_mm, ones_bc, o_ps, shift=True):
    """Softmax normalization for q-block qb.

    The l row (o_ps partition 64) is broadcast across 64 partitions with a
    rank-1 PE matmul (ones[1,64]^T @ l[1,512]) - an on-chip ~1us chain
    instead of a ~8us DRAM round-trip."""
    nc = tc.nc
    F32R = mybir.dt.float32r
    onorm = smsb.tile([128, 512], MMDT, tag="onorm", name=f"onorm_{qb}")
    onorm1 = smsb.tile([64, 512], MMDT, tag="onorm1", name=f"onorm1_{qb}")
    for h in range(2):
        rrow = smsb.tile([65, 512], mybir.dt.float32r, tag="rrow",
                         name=f"rrow_{qb}_{h}")
        nc.vector.tensor_copy(rrow[64:65, :], o_ps[h][64:65, :])
        bps = smsb_mm.tile([128, 512], F32, tag="mm", name=f"bps_{qb}_{h}")
        nc.tensor.matmul(bps[0:64, :], ones_bc[64:65, :], rrow[64:65, :],
                         start=True, stop=True)
        lb = smsb.tile([64, 512], F32, tag="lb", name=f"lb_{qb}_{h}")
        nc.vector.tensor_copy(lb, bps[0:64, :])
        lbi = smsb.tile([64, 512], F32, tag="lbi", name=f"lbi_{qb}_{h}")
        nc.vector.reciprocal_approx_fast(lbi, lb)
        if h == 0:
            nc.vector.tensor_mul(onorm[0:64, :], o_ps[0][0:64, :], lbi)
        else:
            nc.vector.tensor_mul(onorm1, o_ps[1][0:64, :], lbi)
    if shift:
        nc.sync.dma_start(out=onorm[64:128, :], in_=onorm1)
    return onorm, onorm1


def _epilogue_b(tc, qb, mm, smsb, Wp, onorm, y, onorm1=None, Wp2=None):
    """Output projection for q-block qb.  When onorm1/Wp2 are given, the
    contraction is split into per-head halves so the h0 half starts before
    the h1 normalization (and no partition-shift DMA is needed)."""
    nc = tc.nc
    q0 = 512 * qb
    for s in range(4):
        ysb = smsb.tile([128, 2, 512], MMDT, tag="ysb", name=f"ysb_{qb}_{s}")
        for oc in range(2):
            yps = mm.tile([128, 512], F32, tag="mm", name=f"y_{qb}_{s}_{oc}")
            if onorm1 is None:
                nc.tensor.matmul(
                    yps,
                    onorm[:, 128 * s : 128 * (s + 1)],
                    Wp[:, 512 * oc : 512 * (oc + 1)],
                    start=True, stop=True,
                )
            else:
                nc.tensor.matmul(
                    yps,
                    onorm[0:64, 128 * s : 128 * (s + 1)],
                    Wp[0:64, 512 * oc : 512 * (oc + 1)],
                    start=True, stop=False,
                )
                nc.tensor.matmul(
                    yps,
                    onorm1[:, 128 * s : 128 * (s + 1)],
                    Wp2[:, 512 * oc : 512 * (oc + 1)],
                    start=False, stop=True,
                )
            nc.vector.tensor_copy(ysb[:, oc, :], yps)
        nc.sync.dma_start(
            out=y[q0 + 128 * s : q0 + 128 * (s + 1), :],
            in_=ysb,
        )


def _body(tc, xT, wqk, wv, wp, amask, ident, y):
    nc = tc.nc

    persist = tc.alloc_tile_pool(name="persist", bufs=1)

    # Persistent SBUF tensors
    QT = persist.tile([128, N], MMDT, tag="QT")     # [(h,d), n] h0:0..63 h1:64..127
    KT = persist.tile([128, N], MMDT, tag="KT")
    VT = persist.tile([128, N], F32, tag="VT")      # [(h,d), n] pre-transpose
    VP = persist.tile([128, NKT, 2, 65], MMDT, tag="VP")  # [k, kt, h, d|1]
    Wqk = persist.tile([128, NCT, 256], MMDT, tag="Wqk")
    Wv = persist.tile([128, NCT, 128], MMDT, tag="Wv")
    Wp = persist.tile([128, C], MMDT, tag="Wp")
    Wp2 = persist.tile([64, C], MMDT, tag="Wp2")


    amask_sb = None
    if not USE_AFFINE:
        amask_sb = persist.tile([128, 128], F32, tag="amask")
    ident_sb = None
    if not USE_XBAR:
        ident_sb = persist.tile([128, 128], F32, tag="ident")

    # ones columns of V' (index 64 of the last axis)
    ones_st = persist.tile([128, NKT * 2], F32, tag="ones_st")
    nc.vector.memset(ones_st, 1.0)
    nc.vector.tensor_copy(
        VP.rearrange("p t g c -> p (t g) c")[:, :, 64:65],
        ones_st.rearrange("p (n o) -> p n o", o=1),
    )

    ones_bc = persist.tile([65, 64], mybir.dt.float32r, tag="ones_bc")
    nc.vector.tensor_copy(ones_bc, ones_st[0:65, 0:64])

    zero_reg = nc.gpsimd.to_reg(0.0) if USE_AFFINE else None


    with (
        tc.tile_pool(name="xpool", bufs=3) as xpool,
        tc.tile_pool(name="mm", bufs=2, space="PSUM") as mm,
        tc.tile_pool(name="spool", bufs=4, space="PSUM") as spool,
        tc.tile_pool(name="opool", bufs=2, space="PSUM") as opool,
        tc.tile_pool(name="ptpool", bufs=10) as ptpool,
        tc.tile_pool(name="smsb", bufs=6) as smsb,
    ):
        def load_xt(j):
            xt = xpool.tile([128, NCT, 512], MMDT, tag="xt", name=f"xt_{j}")
            nc.sync.dma_start(out=xt, in_=xT[:, j])
            return xt

        xt0 = xpool.tile([128, NCT, 512], MMDT, tag="xt", name="xt_0")
        for q in range(4):
            nc.sync.dma_start(out=Wqk[:, 2 * q : 2 * q + 2],
                              in_=wqk[:, 2 * q : 2 * q + 2])
            nc.sync.dma_start(out=xt0[:, 2 * q : 2 * q + 2],
                              in_=xT[:, 0, 2 * q : 2 * q + 2])
        nc.sync.dma_start(out=Wv, in_=wv)
        if ident_sb is not None:
            nc.sync.dma_start(out=ident_sb, in_=ident)
        if amask_sb is not None:
            nc.sync.dma_start(out=amask_sb, in_=amask)
        nc.sync.dma_start(out=Wp, in_=wp)
        nc.sync.dma_start(out=Wp2, in_=wp[64:128, :])
        # prime the ACT exp table while the loads run
        scr = smsb.tile([1, 8], MMDT, tag="scr", name="scr")
        nc.scalar.activation(scr, ones_st[0:1, 0:8],
                             mybir.ActivationFunctionType.Exp, scale=1.0)

        onorm_prev = None
        for piece in _qkv_pieces(tc, 0, xt0, mm, Wqk, Wv, QT, KT, VT, VP,
                                 ident_sb):
            piece()
        for j in range(NQB):
            # prefetch next x chunk before the epilogue's DMA round-trip
            # lands on the Sync queue, so it transfers during attention
            tail = []
            if j + 1 < NQB:
                xt = load_xt(j + 1)
                tail = _qkv_pieces(tc, j + 1, xt, mm, Wqk, Wv, QT, KT, VT,
                                   VP, ident_sb)

            # previous q-block's projection goes after this block's first
            # group: by then its 1/l round-trip is done and the PE queue
            # never heads on it
            mid = []
            if onorm_prev is not None:
                prev_onorm = onorm_prev
                mid = [lambda pj=j - 1, po=prev_onorm: _epilogue_b(
                    tc, pj, mm, smsb, Wp, po, y)]

            o_ps = [opool.tile([65, 512], F32, tag="o", name=f"o_{j}_{_h}")
                    for _h in range(2)]
            _attention_qb(tc, j, spool, opool, ptpool, QT, KT, VP, o_ps,
                          zero_reg, amask_sb, tail_work=tail, mid_work=mid)
            onorm_prev, onorm1_prev = _epilogue_a(
                tc, j, smsb, mm, ones_bc, o_ps, shift=(j < NQB - 1))

        _epilogue_b(tc, NQB - 1, mm, smsb, Wp, onorm_prev, y,
                    onorm1=onorm1_prev, Wp2=Wp2)

    persist.release()


_NC_CACHE = {}


def _get_nc():
    if "nc" not in _NC_CACHE:
        _NC_CACHE["nc"] = build_nc()
    return _NC_CACHE["nc"]


def make_in_maps(x, w_qkv, w_proj):
    """Host-side sharding: per-core input dicts."""
    from concourse import mybir as _mb
    mdt = _mb.dt.np(MMDT)
    # chunk-major x: [p, j, ct, u] = x[0][512j+u, 128ct+p]
    xTh = np.ascontiguousarray(
        x[0].astype(mdt).reshape(NQB, 512, NCT, 128).transpose(3, 0, 2, 1))
    extras = {}
    if not USE_AFFINE:
        # multiplicative keep-mask for the diagonal 128-band: [kp, qf]
        extras["amask"] = np.ascontiguousarray(np.where(
            np.arange(128)[None, :] >= np.arange(128)[:, None],
            np.float32(1.0), np.float32(0.0)).astype(np.float32))
    if not USE_XBAR:
        extras["ident"] = np.eye(128).astype(np.float32)
    in_maps = []
    for m in range(NCORES):
        r0 = HPC * D * m  # 128*m
        wq = w_qkv[r0 : r0 + 128]
        wk = w_qkv[C + r0 : C + r0 + 128]
        wvm = w_qkv[2 * C + r0 : 2 * C + r0 + 128]
        in_maps.append({
            "xT": xTh,
            "wqk": np.ascontiguousarray(
                np.concatenate([wq, wk], 0).T.astype(mdt)
                .reshape(NCT, 128, 256).transpose(1, 0, 2)),
            "wv": np.ascontiguousarray(
                wvm.T.astype(mdt).reshape(NCT, 128, 128).transpose(1, 0, 2)),
            "wp": np.ascontiguousarray(
                w_proj[:, r0 : r0 + 128].T.astype(mdt)),
            **extras,
        })
    return in_maps


def kernel(x, w_qkv, w_proj, b_proj, _trace=False):
    x = np.asarray(x)
    w_qkv = np.asarray(w_qkv)
    w_proj = np.asarray(w_proj)
    b_proj = np.asarray(b_proj)
    nc = _get_nc()
    in_maps = make_in_maps(x, w_qkv, w_proj)
    res = run_bass_kernel_spmd(
        nc, in_maps, core_ids=list(range(NCORES)), trace=_trace
    )
    out = np.zeros((N, C), dtype=np.float32)
    for r in res.results:
        out += r["y"].astype(np.float32)
    out += b_proj.astype(np.float32)
    out = out.reshape(B, N, C)
    if _trace:
        return out, res
    return out
